# revision 80
# baseline (speedup 1.0000x reference)
"""Trainium2 Bass kernel for nn_CrossHeadDeltaQuantizer.

Sharding: data-parallel over batch (B=8 -> 8 cores, core c owns batch c).

Per-core pipeline; delta heads process 2-block groups ([128 s-rows, 8
tiles, 128 feat] = 4KB/partition) to halve GPSIMD op + semaphore count
and amortize ACT instruction overhead:
  anchor: per-2-block DMA -> ssq (ACT sq-accum j01 / GPSIMD mult + DVE
          reduce j23) -> an/inv -> T(xa) -> data-stationary fp32 mm
          (y, [s,d]) -> normalize (DVE, psum evac) -> staircase via ACT
          Sign + DVE STT combine -> res/alpha/sign -> yh -> T(yh) ->
          data-stationary fp32 mm -> ar
  delta:  d = hd - ar (GPSIMD, [P,8,128]) -> Square bf16 (ACT) +
          seg-reduce dn2 (DVE) -> T(d) per 4-tile half -> dT evac (ACT,
          float32r) -> rotation-stationary float32r mm N=512 (zT, [d,s])
          -> Sign evac bf16 (ACT) -> per-head dn stats -> data-stationary
          bf16 mm (oP, [s,d]) -> out = oP*dnh (DVE) + ar (DVE; split
          DVE/GPSIMD on the last head to thin the pipeline drain)
float32r (~17-bit mantissa, 1 cyc/row at N>=512 vs 4 for fp32) carries
the delta rotation; sign flips from its rounding cost ~0.011 extra L2
relative error, well inside the 2e-2 gate. Loads prefetch 6 groups ahead
of the in-order SP queue so stores never block next-head loads.
"""

import numpy as np
import ml_dtypes

from concourse import bass, bacc, tile, mybir
from concourse.bass_utils import run_bass_kernel_spmd

dt = mybir.dt
Alu = mybir.AluOpType
Act = mybir.ActivationFunctionType

B, H, S, D = 8, 8, 4096, 128
EPS = 1e-8
P = 128
TPB = 4                      # s-tiles per block
NBLK = S // (P * TPB)        # 8
NT = S // P                  # 32 s-tiles per core
NG = NBLK // 2               # 4 two-block groups per head
TPG = 2 * TPB                # 8 s-tiles per group
PREF = 12                    # hd blocks prefetched ahead (groups = PREF // 2)

_CACHE = {}


def _f32(x):
    return float(np.float32(x))


def host_prep(R_anchor, cb_anchor, R_delta, cb_delta):
    R_a = np.ascontiguousarray(np.asarray(R_anchor, np.float32))
    R_d = np.ascontiguousarray(np.asarray(R_delta, np.float32))
    cb = np.asarray(cb_anchor, np.float32)
    cd = np.asarray(cb_delta, np.float32)

    p = {}
    p["r_a_t"] = np.ascontiguousarray(R_a.T)
    p["r_a"] = R_a
    p["r_d_t"] = np.ascontiguousarray(R_d.T)
    p["r_d_bf"] = R_d.astype(ml_dtypes.bfloat16)
    p["ident_f"] = np.eye(P, dtype=np.float32)
    p["ones1"] = np.ones((1, P), dtype=np.float32)
    # row-select one-hots for the non-sym thr partition-broadcast matmul:
    # lhsT slice [:, j*P:(j+1)*P] has row j all-ones -> out = thrs row j
    oh = np.zeros((4, 4 * P), dtype=np.float32)
    for j in range(4):
        oh[j, j * P:(j + 1) * P] = 1.0
    p["onehot4"] = oh

    order = np.argsort(cb, kind="stable")
    cs = cb[order]
    ts_, ge_, dl_ = [], [], []
    for i in range(len(cs) - 1):
        ts_.append(_f32((np.float32(cs[i]) + np.float32(cs[i + 1])) / np.float32(2)))
        ge_.append(bool(order[i + 1] < order[i]))
        dl_.append(_f32(np.float32(cs[i + 1]) - np.float32(cs[i])))
    p["a_ts"], p["a_ge"], p["a_dl"] = ts_, ge_, dl_
    p["a_c0"] = _f32(cs[0])
    # sign-form staircase: q0 = K + sum a_i * sign(y - t_i), a_i = dl_i/2
    amax = max(abs(x) for x in dl_) if dl_ else 1.0
    p["stair_sign_ok"] = all(abs(x) > 1e-6 * amax for x in dl_)
    a_ = [x / 2.0 for x in dl_]
    p["s_a"] = [_f32(x) for x in a_]
    # q0 = [c0+a1+a3] + a1*s1 + dl2*H2 + a3*s3  (s=sign, H=step)
    p["s_K"] = _f32(cs[0] + a_[0] + a_[2])
    if p["stair_sign_ok"]:
        p["s_r12"] = _f32(a_[0] / dl_[1])
        p["s_r23"] = _f32(dl_[1] / a_[2])
        p["s_a3"] = _f32(a_[2])

    c0, c1 = np.float32(cd[0]), np.float32(cd[1])
    k1 = np.float32(2.0) * (c1 - c0)
    k2 = c1 * c1 - c0 * c0
    p["d_m"] = _f32((c0 + c1) / np.float32(2))
    p["d_h_raw"] = _f32((c1 - c0) / np.float32(2))
    p["d_h_eff"] = _f32(((c1 - c0) / np.float32(2)) * np.float32(np.sign(k1) if k1 != 0 else 1.0))
    p["d_k1"] = _f32(k1)
    p["d_k2"] = _f32(k2)
    p["d_sym"] = bool(k2 == np.float32(0.0))
    return p


def _sqrt_refined(nc, pool, q_ap, out_ap, n, steps=1):
    """out = sqrt(q) — ACT Sqrt seed + Newton refinement (ACT sqrt has a loose
    ULP budget). q_ap: [P, n] fp32 SBUF, must be >= 1e-35 (caller guards)."""
    s = out_ap
    r = pool.tile([P, n], dt.float32, tag="st_r")
    t0 = pool.tile([P, n], dt.float32, tag="st_t0")
    nc.scalar.activation(out=s, in_=q_ap, func=Act.Sqrt)
    for _ in range(steps):
        nc.vector.tensor_scalar(out=r, in0=s, scalar1=1e-30, scalar2=None,
                                op0=Alu.max)
        nc.vector.reciprocal(out=r, in_=r)
        nc.vector.tensor_tensor(out=t0, in0=q_ap, in1=r, op=Alu.mult)   # q/s
        nc.vector.tensor_tensor(out=t0, in0=t0, in1=s, op=Alu.add)      # s + q/s
        nc.vector.tensor_scalar(out=s, in0=t0, scalar1=0.5, scalar2=None,
                                op0=Alu.mult)
    return s


def build(p):
    nc = bacc.Bacc()
    kv = nc.declare_dram_parameter("kv", [H, S, D], dt.float32, isOutput=False)
    r_a_t_d = nc.declare_dram_parameter("r_a_t", [D, D], dt.float32, isOutput=False)
    r_a_d = nc.declare_dram_parameter("r_a", [D, D], dt.float32, isOutput=False)
    r_d_t_d = nc.declare_dram_parameter("r_d_t", [D, D], dt.float32, isOutput=False)
    r_d_bf_d = nc.declare_dram_parameter("r_d_bf", [D, D], dt.bfloat16, isOutput=False)
    id_f_d = nc.declare_dram_parameter("ident_f", [P, P], dt.float32, isOutput=False)
    ones1_d = nc.declare_dram_parameter("ones1", [1, P], dt.float32, isOutput=False)
    onehot4_d = nc.declare_dram_parameter("onehot4", [4, 4 * P], dt.float32,
                                          isOutput=False)
    out = nc.declare_dram_parameter("out", [H, S, D], dt.float32, isOutput=True)

    sym = p["d_sym"]
    sign_stair = p["stair_sign_ok"]

    if sign_stair:
        # register const APs for the staircase Sign biases
        for tt in p["a_ts"]:
            v = _f32(-tt)
            if (dt.float32, v) not in nc.const_aps.aps:
                t_ = nc.alloc_sbuf_tensor(f"const-f32-{v}", [128, 1], dt.float32)
                nc.gpsimd.memset(t_.ap(), v)
                nc.const_aps.aps[(dt.float32, v)] = t_.ap()
        nc.all_engine_barrier()

    def head_dram(t, h):
        # partition p holds rows [NT*p, NT*p+NT): one 16KB contiguous run
        return t[h].rearrange("(p j) d -> p j d", p=P)

    with tile.TileContext(nc) as tc:
        with tc.tile_pool(name="consts", bufs=1) as cpool, \
             tc.tile_pool(name="stats", bufs=1) as spool, \
             tc.tile_pool(name="hstats", bufs=(3 if p["d_sym"] else 2)) as hpool, \
             tc.tile_pool(name="resid", bufs=1) as rpool, \
             tc.tile_pool(name="sres", bufs=(3 if p["d_sym"] else 2)) as stpool, \
             tc.tile_pool(name="io", bufs=PREF // 2 + 2) as iopool, \
             tc.tile_pool(name="ob", bufs=4) as obpool, \
             tc.tile_pool(name="work", bufs=2) as wpool, \
             tc.tile_pool(name="yh", bufs=(NBLK if p["d_sym"] else 3)) as ypool, \
             tc.tile_pool(name="junk", bufs=2) as jpool, \
             tc.tile_pool(name="dpool", bufs=3) as dpool, \
             tc.tile_pool(name="dtp", bufs=3) as dtpool, \
             tc.tile_pool(name="ps", bufs=1, space="PSUM") as psum:

            # ---- constants ----
            r_a_t = cpool.tile([D, D], dt.float32, tag="c_rat")
            r_a = cpool.tile([D, D], dt.float32, tag="c_ra")
            r_d_t = cpool.tile([D, D], dt.float32, tag="c_rdt")
            r_d_bf = cpool.tile([D, D], dt.bfloat16, tag="c_rdbf")
            id_f = cpool.tile([P, P], dt.float32, tag="c_idf")
            ones1 = cpool.tile([1, P], dt.float32, tag="c_ones")
            onehot4 = cpool.tile([4, 4 * P], dt.float32, tag="c_oh4")
            for t_, d_ in ((r_a_t, r_a_t_d), (r_a, r_a_d), (r_d_t, r_d_t_d),
                           (r_d_bf, r_d_bf_d), (id_f, id_f_d), (ones1, ones1_d),
                           (onehot4, onehot4_d)):
                nc.sync.dma_start(out=t_, in_=d_[:])
            # fp32r copy of R_d^T for the fast (1 cyc/row) rotation matmul
            r_d_t_r = cpool.tile([D, D], dt.float32r, tag="c_rdtr")
            nc.scalar.activation(out=r_d_t_r, in_=r_d_t, func=Act.Copy)


            # ---- anchor stats tiles ----
            ssq = spool.tile([P, NT], dt.float32, tag="ssq")
            an = spool.tile([P, NT], dt.float32, tag="an")
            inv = spool.tile([P, NT], dt.float32, tag="inv")
            alph = spool.tile([P, NT], dt.float32, tag="alph")

            # ---- resident tensors (whole-head) ----
            xah = rpool.tile([P, NT, P], dt.float32, tag="xah", name="xah")
            arh = rpool.tile([P, NT, P], dt.float32, tag="arh", name="arh")
            xa = [xah[:, b * TPB:(b + 1) * TPB] for b in range(NBLK)]
            ar = [arh[:, b * TPB:(b + 1) * TPB] for b in range(NBLK)]

            # per-2-block-group hd loads (4KB/partition contiguous),
            # prefetched PREF//2 groups ahead on SP
            hd_tiles = {}

            def load_hd(h, g):
                t_ = iopool.tile([P, TPG, P], dt.float32, tag="hd",
                                 name=f"hd{h}_{g}")
                # loads issue from the ACT queue: they never wait on data,
                # so they bypass stores stalled in the Sync queue, and they
                # stay off GPSIMD whose subs head the delta chain
                nc.scalar.dma_start(
                    out=t_, in_=head_dram(kv, h)[:, g * TPG:(g + 1) * TPG])
                hd_tiles[(h, g)] = t_

            # ================= ANCHOR =================
            # per-2-block anchor loads so ssq starts as soon as data lands
            for g in range(NBLK // 2):
                s2 = slice(g * 2 * TPB, (g + 1) * 2 * TPB)
                nc.scalar.dma_start(out=xah[:, s2], in_=head_dram(kv, 0)[:, s2])
            for b in range(NBLK):
                junk = jpool.tile([P, TPB, P], dt.float32, tag="junkA")
                for j in range(2):
                    t = b * TPB + j
                    nc.scalar.activation(out=junk[:, j], in_=xa[b][:, j],
                                         func=Act.Square,
                                         accum_out=ssq[:, t:t + 1])
                # j=2,3 squared + reduced on GPSIMD to unload ACT and DVE
                nc.gpsimd.tensor_tensor(out=junk[:, 2:], in0=xa[b][:, 2:],
                                        in1=xa[b][:, 2:], op=Alu.mult)
                nc.vector.tensor_reduce(out=ssq[:, b * TPB + 2:b * TPB + 4],
                                        in_=junk[:, 2:],
                                        axis=mybir.AxisListType.X, op=Alu.add)
            # prefetch the first PREF delta-head blocks while anchor computes
            for i in range(PREF // 2):
                load_hd(1 + i // NG, i % NG)
            # stats: an = sqrt(ssq) (Newton-refined), inv = 1/(an+EPS)
            nc.vector.tensor_scalar(out=inv, in0=ssq, scalar1=1e-35, scalar2=None,
                                    op0=Alu.max)
            _sqrt_refined(nc, spool, inv, an, NT)
            nc.vector.tensor_scalar(out=inv, in0=an, scalar1=_f32(EPS), scalar2=None,
                                    op0=Alu.add)
            nc.vector.reciprocal(out=inv, in_=inv)

            yhr = {}

            # ---- anchor: fwd+quantize (DVE-heavy) interleaved with
            # bwd (PE-heavy) at a 2-block stagger ----
            def anchor_fwd(b):
                sl = slice(b * TPB, (b + 1) * TPB)
                pXa = psum.tile([P, TPB, P], dt.float32, tag="psT", bufs=2,
                                name="pXa")
                for j in range(TPB):
                    nc.tensor.transpose(pXa[:, j], xa[b][:, j], id_f)
                xaT = wpool.tile([P, TPB, P], dt.float32, tag="xaT")
                nc.scalar.activation(out=xaT, in_=pXa, func=Act.Copy)
                yP = psum.tile([P, TPB, P], dt.float32, tag="psM",
                               bufs=(2 if sym else 1), name="yP")
                for j in range(TPB):
                    nc.tensor.matmul(yP[:, j], lhsT=xaT[:, j], rhs=r_a_t,
                                     start=True, stop=True)
                yt = wpool.tile([P, TPB, P], dt.float32, tag="yt")
                inv_bc = inv[:, sl][:, :, None].broadcast_to([P, TPB, P])
                nc.vector.tensor_tensor(out=yt, in0=yP, in1=inv_bc, op=Alu.mult)
                q0 = wpool.tile([P, TPB, P], dt.float32, tag="q0")
                if sign_stair:
                    # q0 = K + a1*s1 + a2*s2 + dl3*H3
                    s1 = wpool.tile([P, TPB, P], dt.float32, tag="a1")
                    s2 = wpool.tile([P, TPB, P], dt.float32, tag="a2")
                    h3 = wpool.tile([P, TPB, P], dt.float32, tag="a3")
                    nc.scalar.activation(out=s1, in_=yt, func=Act.Sign,
                                         bias=_f32(-p["a_ts"][0]))
                    nc.vector.tensor_scalar(
                        out=s2, in0=yt, scalar1=p["a_ts"][1], scalar2=None,
                        op0=(Alu.is_ge if p["a_ge"][1] else Alu.is_gt))
                    nc.scalar.activation(out=h3, in_=yt, func=Act.Sign,
                                         bias=_f32(-p["a_ts"][2]))
                    u = wpool.tile([P, TPB, P], dt.float32, tag="u")
                    nc.vector.scalar_tensor_tensor(out=u, in0=s1,
                                                   scalar=p["s_r12"], in1=s2,
                                                   op0=Alu.mult, op1=Alu.add)
                    nc.vector.scalar_tensor_tensor(out=u, in0=u,
                                                   scalar=p["s_r23"], in1=h3,
                                                   op0=Alu.mult, op1=Alu.add)
                    nc.scalar.activation(out=q0, in_=u, func=Act.Copy,
                                         scale=p["s_a3"], bias=p["s_K"])
                else:
                    a1 = wpool.tile([P, TPB, P], dt.float32, tag="a1")
                    a2 = wpool.tile([P, TPB, P], dt.float32, tag="a2")
                    a3 = wpool.tile([P, TPB, P], dt.float32, tag="a3")
                    stair = list(zip(p["a_ts"], p["a_ge"], p["a_dl"]))
                    for ai, (tt, ge, dl) in zip((a1, a2, a3), stair):
                        nc.vector.tensor_scalar(out=ai, in0=yt, scalar1=tt,
                                                scalar2=dl,
                                                op0=(Alu.is_ge if ge else Alu.is_gt),
                                                op1=Alu.mult)
                    nc.vector.scalar_tensor_tensor(out=q0, in0=a1,
                                                   scalar=p["a_c0"], in1=a2,
                                                   op0=Alu.add, op1=Alu.add)
                    nc.vector.tensor_tensor(out=q0, in0=q0, in1=a3, op=Alu.add)
                res = wpool.tile([P, TPB, P], dt.float32, tag="res")
                nc.gpsimd.tensor_tensor(out=res, in0=yt, in1=q0, op=Alu.subtract)
                nc.vector.tensor_reduce(out=alph[:, sl], in_=res,
                                        axis=mybir.AxisListType.X, op=Alu.add,
                                        apply_absolute_value=True)
                nc.vector.tensor_scalar(out=alph[:, sl], in0=alph[:, sl],
                                        scalar1=_f32(1.0 / D), scalar2=None,
                                        op0=Alu.mult)
                sgn = wpool.tile([P, TPB, P], dt.float32, tag="sgn")
                nc.scalar.activation(out=sgn, in_=res, func=Act.Sign)
                al_bc = alph[:, sl][:, :, None].broadcast_to([P, TPB, P])
                nc.gpsimd.tensor_tensor(out=sgn, in0=sgn, in1=al_bc, op=Alu.mult)
                yhr[b] = ypool.tile([P, TPB, P], dt.float32, tag="yh",
                                    name=f"yh{b}")
                nc.gpsimd.tensor_tensor(out=yhr[b], in0=sgn, in1=q0, op=Alu.add)

            def anchor_bwd(b):
                sl = slice(b * TPB, (b + 1) * TPB)
                pYh = psum.tile([P, TPB, P], dt.float32, tag="psT", bufs=2,
                                name="pYh")
                for j in range(TPB):
                    nc.tensor.transpose(pYh[:, j], yhr[b][:, j], id_f)
                del yhr[b]
                yhT = wpool.tile([P, TPB, P], dt.float32, tag="yhT")
                nc.scalar.activation(out=yhT, in_=pYh, func=Act.Copy)
                arP = psum.tile([P, TPB, P], dt.float32, tag="psM",
                                bufs=(2 if sym else 1), name="arP")
                for j in range(TPB):
                    nc.tensor.matmul(arP[:, j], lhsT=yhT[:, j], rhs=r_a,
                                     start=True, stop=True)
                an_bc = an[:, sl][:, :, None].broadcast_to([P, TPB, P])
                nc.vector.tensor_tensor(out=ar[b], in0=arP, in1=an_bc,
                                        op=Alu.mult)
                nc.sync.dma_start(out=head_dram(out, 0)[:, sl], in_=ar[b])

            # ================= DELTA HEADS =================
            # software-pipelined: pass1 of head h+1 interleaves with pass2 of
            # head h at block granularity so ACT/DVE/Pool queues stay fed
            # across the per-head stats barrier.
            nload = NG * (H - 1)

            def load_idx(i):
                if i < nload:
                    load_hd(1 + i // NG, i % NG)

            st = {}

            def head_state(h):
                st[h] = dict(
                    dn2h=hpool.tile([P, NT], dt.bfloat16, tag="dn2h",
                                    name=f"dn2h{h}"),
                    sT=([stpool.tile([P, TPG, P], dt.bfloat16, tag=f"sT{g}",
                                     name=f"sT{h}_{g}") for g in range(NG)]
                        if sym else None),
                    sres=[None] * NG,
                )

            def pass1(h, g):
                v = st[h]
                sl2 = slice(g * TPG, (g + 1) * TPG)
                load_idx((h - 1) * NG + g + PREF // 2)
                hd = hd_tiles.pop((h, g))
                d_t = dpool.tile([P, TPG, P], dt.float32, tag="d")
                nc.gpsimd.tensor_tensor(out=d_t, in0=hd, in1=arh[:, sl2],
                                        op=Alu.subtract)
                dsq2 = jpool.tile([P, TPG, P], dt.bfloat16, tag="dsq2")
                nc.scalar.activation(out=dsq2, in_=d_t, func=Act.Square)
                with nc.allow_low_precision("dn2 bf16 seg-reduce"):
                    nc.vector.tensor_reduce(out=v["dn2h"][:, sl2], in_=dsq2,
                                            axis=mybir.AxisListType.X,
                                            op=Alu.add)
                dT = dtpool.tile([P, TPG, P], dt.float32r, tag="dT")
                if not sym:
                    v["sres"][g] = stpool.tile([P, TPG, P], dt.float32,
                                               tag=f"sr{g}", name=f"sr{h}_{g}")
                for half in range(2):
                    hs = slice(half * TPB, (half + 1) * TPB)
                    pTd = psum.tile([P, TPB, P], dt.float32, tag="psT2",
                                    bufs=(2 if sym else 1), name="pTd")
                    for j in range(TPB):
                        nc.tensor.transpose(pTd[:, j], d_t[:, half * TPB + j],
                                            id_f)
                    nc.scalar.activation(out=dT[:, hs], in_=pTd, func=Act.Copy)
                    zP = psum.tile([P, TPB, P], dt.float32, tag="psM",
                                   bufs=(2 if sym else 1), name="zP")
                    nc.tensor.matmul(zP.rearrange("p j d -> p (j d)"),
                                     lhsT=r_d_t_r,
                                     rhs=dT[:, hs].rearrange("p j d -> p (j d)"),
                                     start=True, stop=True)
                    if sym:
                        nc.scalar.activation(out=v["sT"][g][:, hs], in_=zP,
                                             func=Act.Sign)
                    else:
                        nc.scalar.activation(out=v["sres"][g][:, hs], in_=zP,
                                             func=Act.Copy)

            def head_stats(h):
                v = st[h]
                dnm = hpool.tile([P, NT], dt.float32, tag="dnm", name=f"dnm{h}")
                dn_t = hpool.tile([P, NT], dt.float32, tag="dn_t", name=f"dn{h}")
                dnh = hpool.tile([P, NT], dt.float32, tag="dnh", name=f"dnh{h}")
                nc.vector.tensor_scalar(out=dnm, in0=v["dn2h"], scalar1=1e-35,
                                        scalar2=None, op0=Alu.max)
                _sqrt_refined(nc, hpool, dnm, dn_t, NT)
                if sym:
                    nc.vector.tensor_scalar(out=dnh, in0=dn_t,
                                            scalar1=p["d_h_eff"],
                                            scalar2=None, op0=Alu.mult)
                else:
                    # thr = k2*(dn+eps); sign(z*k1 - thr); scale recon by dn
                    thr = hpool.tile([P, NT], dt.float32, tag="thr",
                                     name=f"thr{h}")
                    nc.vector.tensor_scalar(out=thr, in0=dn_t,
                                            scalar1=_f32(EPS),
                                            scalar2=_f32(p["d_k2"]),
                                            op0=Alu.add, op1=Alu.mult)
                    nc.vector.tensor_scalar(out=dnh, in0=dn_t,
                                            scalar1=_f32(p["d_h_raw"]),
                                            scalar2=None, op0=Alu.mult)
                    v["thr"] = thr
                v["dn_t"], v["dnh"] = dn_t, dnh

            def pass2(h, g):
                v = st[h]
                sl2 = slice(g * TPG, (g + 1) * TPG)
                drain = (h == H - 1)
                ob2 = obpool.tile([P, TPG, P], dt.float32, tag="ob2",
                                  name=f"ob2_{h}_{g}")
                for half in range(2):
                    hs = slice(half * TPB, (half + 1) * TPB)
                    slh = slice(g * TPG + half * TPB,
                                g * TPG + (half + 1) * TPB)
                    if not sym:
                        # broadcast thr along partitions via K=1 matmul, then
                        # sign(z*k1 - thr) per tile (cold path)
                        pThr = psum.tile([TPB, P], dt.float32, tag="psS",
                                         bufs=1, name="pThr")
                        nc.tensor.transpose(pThr, v["thr"][:, slh], id_f)
                        thrs = wpool.tile([TPB, P], dt.float32, tag="thrs")
                        nc.scalar.activation(out=thrs, in_=pThr, func=Act.Copy)
                        sg = wpool.tile([P, TPB, P], dt.float32, tag="sg")
                        for j in range(TPB):
                            thrB = psum.tile([P, P], dt.float32, tag="psB",
                                             bufs=1, name="thrB")
                            nc.tensor.matmul(thrB,
                                             lhsT=onehot4[:, j * P:(j + 1) * P],
                                             rhs=thrs,
                                             start=True, stop=True)
                            nc.vector.scalar_tensor_tensor(
                                out=sg[:, j],
                                in0=v["sres"][g][:, half * TPB + j],
                                scalar=_f32(p["d_k1"]), in1=thrB,
                                op0=Alu.mult, op1=Alu.subtract)
                        qsc = wpool.tile([P, TPB, P], dt.bfloat16, tag="qsc")
                        nc.scalar.activation(out=qsc, in_=sg, func=Act.Sign)
                        # q = m + h_raw * s  (bf16), recon scaled by dn
                        nc.vector.tensor_scalar(out=qsc, in0=qsc,
                                                scalar1=_f32(p["d_h_raw"]),
                                                scalar2=_f32(p["d_m"]),
                                                op0=Alu.mult, op1=Alu.add)
                        qsrc, scale_t = qsc, v["dn_t"]
                        qsl = slice(0, TPB)
                    else:
                        qsrc, scale_t = v["sT"][g], v["dnh"]
                        qsl = hs
                    oP = psum.tile([P, TPB, P], dt.float32, tag="psO",
                                   bufs=2, name="oP")
                    for j in range(TPB):
                        nc.tensor.matmul(oP[:, j],
                                         lhsT=qsrc[:, qsl][:, j],
                                         rhs=r_d_bf, start=True, stop=True)
                    sc_bc = scale_t[:, slh][:, :, None].broadcast_to(
                        [P, TPB, P])
                    nc.vector.tensor_tensor(out=ob2[:, hs], in0=oP, in1=sc_bc,
                                            op=Alu.mult)
                if drain:
                    # split the +anchor add DVE/GPSIMD to thin the tail
                    nc.vector.tensor_tensor(out=ob2[:, :TPB],
                                            in0=ob2[:, :TPB],
                                            in1=arh[:, sl2][:, :TPB],
                                            op=Alu.add)
                    nc.gpsimd.tensor_tensor(out=ob2[:, TPB:],
                                            in0=ob2[:, TPB:],
                                            in1=arh[:, sl2][:, TPB:],
                                            op=Alu.add)
                else:
                    nc.vector.tensor_tensor(out=ob2, in0=ob2,
                                            in1=arh[:, sl2], op=Alu.add)
                nc.sync.dma_start(out=head_dram(out, h)[:, sl2], in_=ob2)

            head_state(1)
            for b in range(NBLK):
                anchor_fwd(b)
            for b in range(NBLK):
                anchor_bwd(b)
            for g in range(NG):
                pass1(1, g)
            head_stats(1)
            # head h+1's pass1 + stats emit before head h's pass2 batch, so
            # the stats chain latency hides under pass2 instead of stalling
            # the next head's output evacuation (psO backpressure -> PE).
            for h in range(1, H):
                if h + 1 < H:
                    head_state(h + 1)
                    for g in range(NG):
                        pass1(h + 1, g)
                    head_stats(h + 1)
                for g in range(NG):
                    pass2(h, g)
                del st[h]
    nc.finalize()
    return nc


SHARED_KEYS = ("r_a_t", "r_a", "r_d_t", "r_d_bf", "ident_f", "ones1", "onehot4")


def core_inputs(shared, kv_states, c):
    return dict(shared, kv=kv_states[c])


def assemble_output(results):
    return np.stack([results[c]["out"] for c in range(B)], axis=0).astype(np.float32)


def kernel(**inputs):
    kv_states = np.ascontiguousarray(np.asarray(inputs["kv_states"], np.float32))
    p = host_prep(inputs["R_anchor"], inputs["cb_anchor"],
                  inputs["R_delta"], inputs["cb_delta"])
    key = (p["d_sym"], p["stair_sign_ok"], tuple(p["a_ts"]), tuple(p["a_ge"]),
           tuple(p["a_dl"]), p["a_c0"], p["d_m"], p["d_h_eff"], p["d_k1"],
           p["d_k2"])
    if key not in _CACHE:
        _CACHE[key] = build(p)
    nc = _CACHE[key]

    shared = {k: p[k] for k in SHARED_KEYS}
    in_maps = [core_inputs(shared, kv_states, c) for c in range(B)]
    res = run_bass_kernel_spmd(nc, in_maps, core_ids=list(range(B)))
    return assemble_output(res.results)


if __name__ == "__main__":
    rng = np.random.default_rng(0)
    fake = {
        "kv_states": rng.standard_normal((B, H, S, D), dtype=np.float32),
        "R_anchor": rng.standard_normal((D, D), dtype=np.float32),
        "cb_anchor": np.sort(rng.standard_normal(4).astype(np.float32)),
        "R_delta": rng.standard_normal((D, D), dtype=np.float32),
        "cb_delta": np.sort(rng.standard_normal(2).astype(np.float32)),
    }
    o = kernel(**fake)
    print("ran", o.shape, o.dtype)



# revision 81
# speedup vs baseline: 1.0052x; 1.0052x over previous
"""Trainium2 Bass kernel for nn_CrossHeadDeltaQuantizer.

Sharding: data-parallel over batch (B=8 -> 8 cores, core c owns batch c).

Per-core pipeline; delta heads process 2-block groups ([128 s-rows, 8
tiles, 128 feat] = 4KB/partition) to halve GPSIMD op + semaphore count
and amortize ACT instruction overhead:
  anchor: per-2-block DMA -> ssq (ACT sq-accum j01 / GPSIMD mult + DVE
          reduce j23) -> an/inv -> T(xa) -> data-stationary fp32 mm
          (y, [s,d]) -> normalize (DVE, psum evac) -> staircase via ACT
          Sign + DVE STT combine -> res/alpha/sign -> yh -> T(yh) ->
          data-stationary fp32 mm -> ar
  delta:  d = hd - ar (GPSIMD, [P,8,128]) -> Square bf16 (ACT) +
          seg-reduce dn2 (DVE) -> T(d) per 4-tile half -> dT evac (ACT,
          float32r) -> rotation-stationary float32r mm N=512 (zT, [d,s])
          -> Sign evac bf16 (ACT) -> per-head dn stats -> data-stationary
          bf16 mm (oP, [s,d]) -> out = oP*dnh (DVE) + ar (DVE; split
          DVE/GPSIMD on the last head to thin the pipeline drain)
float32r (~17-bit mantissa, 1 cyc/row at N>=512 vs 4 for fp32) carries
the delta rotation; sign flips from its rounding cost ~0.011 extra L2
relative error, well inside the 2e-2 gate. Loads prefetch 6 groups ahead
of the in-order SP queue so stores never block next-head loads.
"""

import numpy as np
import ml_dtypes

from concourse import bass, bacc, tile, mybir
from concourse.bass_utils import run_bass_kernel_spmd

dt = mybir.dt
Alu = mybir.AluOpType
Act = mybir.ActivationFunctionType

B, H, S, D = 8, 8, 4096, 128
EPS = 1e-8
P = 128
TPB = 4                      # s-tiles per block
NBLK = S // (P * TPB)        # 8
NT = S // P                  # 32 s-tiles per core
NG = NBLK // 2               # 4 two-block groups per head
TPG = 2 * TPB                # 8 s-tiles per group
PREF = 12                    # hd blocks prefetched ahead (groups = PREF // 2)

_CACHE = {}


def _f32(x):
    return float(np.float32(x))


def host_prep(R_anchor, cb_anchor, R_delta, cb_delta):
    R_a = np.ascontiguousarray(np.asarray(R_anchor, np.float32))
    R_d = np.ascontiguousarray(np.asarray(R_delta, np.float32))
    cb = np.asarray(cb_anchor, np.float32)
    cd = np.asarray(cb_delta, np.float32)

    p = {}
    p["r_a_t"] = np.ascontiguousarray(R_a.T)
    p["r_a"] = R_a
    p["r_d_t"] = np.ascontiguousarray(R_d.T)
    p["r_d_bf"] = R_d.astype(ml_dtypes.bfloat16)
    p["ident_f"] = np.eye(P, dtype=np.float32)
    p["ones1"] = np.ones((1, P), dtype=np.float32)
    # row-select one-hots for the non-sym thr partition-broadcast matmul:
    # lhsT slice [:, j*P:(j+1)*P] has row j all-ones -> out = thrs row j
    oh = np.zeros((4, 4 * P), dtype=np.float32)
    for j in range(4):
        oh[j, j * P:(j + 1) * P] = 1.0
    p["onehot4"] = oh

    order = np.argsort(cb, kind="stable")
    cs = cb[order]
    ts_, ge_, dl_ = [], [], []
    for i in range(len(cs) - 1):
        ts_.append(_f32((np.float32(cs[i]) + np.float32(cs[i + 1])) / np.float32(2)))
        ge_.append(bool(order[i + 1] < order[i]))
        dl_.append(_f32(np.float32(cs[i + 1]) - np.float32(cs[i])))
    p["a_ts"], p["a_ge"], p["a_dl"] = ts_, ge_, dl_
    p["a_c0"] = _f32(cs[0])
    # sign-form staircase: q0 = K + sum a_i * sign(y - t_i), a_i = dl_i/2
    amax = max(abs(x) for x in dl_) if dl_ else 1.0
    p["stair_sign_ok"] = all(abs(x) > 1e-6 * amax for x in dl_)
    a_ = [x / 2.0 for x in dl_]
    p["s_a"] = [_f32(x) for x in a_]
    # q0 = [c0+a1+a3] + a1*s1 + dl2*H2 + a3*s3  (s=sign, H=step)
    p["s_K"] = _f32(cs[0] + a_[0] + a_[2])
    if p["stair_sign_ok"]:
        p["s_r12"] = _f32(a_[0] / dl_[1])
        p["s_r23"] = _f32(dl_[1] / a_[2])
        p["s_a3"] = _f32(a_[2])

    c0, c1 = np.float32(cd[0]), np.float32(cd[1])
    k1 = np.float32(2.0) * (c1 - c0)
    k2 = c1 * c1 - c0 * c0
    p["d_m"] = _f32((c0 + c1) / np.float32(2))
    p["d_h_raw"] = _f32((c1 - c0) / np.float32(2))
    p["d_h_eff"] = _f32(((c1 - c0) / np.float32(2)) * np.float32(np.sign(k1) if k1 != 0 else 1.0))
    p["d_k1"] = _f32(k1)
    p["d_k2"] = _f32(k2)
    p["d_sym"] = bool(k2 == np.float32(0.0))
    return p


def _sqrt_refined(nc, pool, q_ap, out_ap, n, steps=1):
    """out = sqrt(q) — ACT Sqrt seed + Newton refinement (ACT sqrt has a loose
    ULP budget). q_ap: [P, n] fp32 SBUF, must be >= 1e-35 (caller guards)."""
    s = out_ap
    r = pool.tile([P, n], dt.float32, tag="st_r")
    t0 = pool.tile([P, n], dt.float32, tag="st_t0")
    nc.scalar.activation(out=s, in_=q_ap, func=Act.Sqrt)
    for _ in range(steps):
        nc.vector.tensor_scalar(out=r, in0=s, scalar1=1e-30, scalar2=None,
                                op0=Alu.max)
        nc.vector.reciprocal(out=r, in_=r)
        nc.vector.tensor_tensor(out=t0, in0=q_ap, in1=r, op=Alu.mult)   # q/s
        nc.vector.tensor_tensor(out=t0, in0=t0, in1=s, op=Alu.add)      # s + q/s
        nc.vector.tensor_scalar(out=s, in0=t0, scalar1=0.5, scalar2=None,
                                op0=Alu.mult)
    return s


def build(p):
    nc = bacc.Bacc()
    kv = nc.declare_dram_parameter("kv", [H, S, D], dt.float32, isOutput=False)
    r_a_t_d = nc.declare_dram_parameter("r_a_t", [D, D], dt.float32, isOutput=False)
    r_a_d = nc.declare_dram_parameter("r_a", [D, D], dt.float32, isOutput=False)
    r_d_t_d = nc.declare_dram_parameter("r_d_t", [D, D], dt.float32, isOutput=False)
    r_d_bf_d = nc.declare_dram_parameter("r_d_bf", [D, D], dt.bfloat16, isOutput=False)
    id_f_d = nc.declare_dram_parameter("ident_f", [P, P], dt.float32, isOutput=False)
    ones1_d = nc.declare_dram_parameter("ones1", [1, P], dt.float32, isOutput=False)
    onehot4_d = nc.declare_dram_parameter("onehot4", [4, 4 * P], dt.float32,
                                          isOutput=False)
    out = nc.declare_dram_parameter("out", [H, S, D], dt.float32, isOutput=True)

    sym = p["d_sym"]
    sign_stair = p["stair_sign_ok"]

    if sign_stair:
        # register const APs for the staircase Sign biases
        for tt in p["a_ts"]:
            v = _f32(-tt)
            if (dt.float32, v) not in nc.const_aps.aps:
                t_ = nc.alloc_sbuf_tensor(f"const-f32-{v}", [128, 1], dt.float32)
                nc.gpsimd.memset(t_.ap(), v)
                nc.const_aps.aps[(dt.float32, v)] = t_.ap()
        nc.all_engine_barrier()

    def head_dram(t, h):
        # partition p holds rows [NT*p, NT*p+NT): one 16KB contiguous run
        return t[h].rearrange("(p j) d -> p j d", p=P)

    with tile.TileContext(nc) as tc:
        with tc.tile_pool(name="consts", bufs=1) as cpool, \
             tc.tile_pool(name="stats", bufs=1) as spool, \
             tc.tile_pool(name="hstats", bufs=(3 if p["d_sym"] else 2)) as hpool, \
             tc.tile_pool(name="resid", bufs=1) as rpool, \
             tc.tile_pool(name="sres", bufs=(3 if p["d_sym"] else 2)) as stpool, \
             tc.tile_pool(name="io", bufs=PREF // 2 + 2) as iopool, \
             tc.tile_pool(name="ob", bufs=4) as obpool, \
             tc.tile_pool(name="work", bufs=2) as wpool, \
             tc.tile_pool(name="yh", bufs=(NBLK if p["d_sym"] else 3)) as ypool, \
             tc.tile_pool(name="junk", bufs=2) as jpool, \
             tc.tile_pool(name="dpool", bufs=3) as dpool, \
             tc.tile_pool(name="dtp", bufs=3) as dtpool, \
             tc.tile_pool(name="ps", bufs=1, space="PSUM") as psum:

            # ---- constants ----
            r_a_t = cpool.tile([D, D], dt.float32, tag="c_rat")
            r_a = cpool.tile([D, D], dt.float32, tag="c_ra")
            r_d_t = cpool.tile([D, D], dt.float32, tag="c_rdt")
            r_d_bf = cpool.tile([D, D], dt.bfloat16, tag="c_rdbf")
            id_f = cpool.tile([P, P], dt.float32, tag="c_idf")
            ones1 = cpool.tile([1, P], dt.float32, tag="c_ones")
            onehot4 = cpool.tile([4, 4 * P], dt.float32, tag="c_oh4")
            for t_, d_ in ((r_a_t, r_a_t_d), (r_a, r_a_d), (r_d_t, r_d_t_d),
                           (r_d_bf, r_d_bf_d), (id_f, id_f_d), (ones1, ones1_d),
                           (onehot4, onehot4_d)):
                nc.sync.dma_start(out=t_, in_=d_[:])
            # fp32r copy of R_d^T for the fast (1 cyc/row) rotation matmul
            r_d_t_r = cpool.tile([D, D], dt.float32r, tag="c_rdtr")
            nc.scalar.activation(out=r_d_t_r, in_=r_d_t, func=Act.Copy)


            # ---- anchor stats tiles ----
            ssq = spool.tile([P, NT], dt.float32, tag="ssq")
            an = spool.tile([P, NT], dt.float32, tag="an")
            inv = spool.tile([P, NT], dt.float32, tag="inv")
            alph = spool.tile([P, NT], dt.float32, tag="alph")

            # ---- resident tensors (whole-head) ----
            xah = rpool.tile([P, NT, P], dt.float32, tag="xah", name="xah")
            arh = rpool.tile([P, NT, P], dt.float32, tag="arh", name="arh")
            xa = [xah[:, b * TPB:(b + 1) * TPB] for b in range(NBLK)]
            ar = [arh[:, b * TPB:(b + 1) * TPB] for b in range(NBLK)]

            # per-2-block-group hd loads (4KB/partition contiguous),
            # prefetched PREF//2 groups ahead on SP
            hd_tiles = {}

            def load_hd(h, g):
                t_ = iopool.tile([P, TPG, P], dt.float32, tag="hd",
                                 name=f"hd{h}_{g}")
                # alternate load issues between the GPSIMD queue (never
                # stalls on data) and Sync, halving GPSIMD's descriptor-gen
                # load; the prefetch cushion absorbs Sync store-blocking
                eng = nc.gpsimd if (h * NG + g) % 2 == 0 else nc.sync
                eng.dma_start(
                    out=t_, in_=head_dram(kv, h)[:, g * TPG:(g + 1) * TPG])
                hd_tiles[(h, g)] = t_

            # ================= ANCHOR =================
            # per-2-block anchor loads so ssq starts as soon as data lands
            for g in range(NBLK // 2):
                s2 = slice(g * 2 * TPB, (g + 1) * 2 * TPB)
                nc.gpsimd.dma_start(out=xah[:, s2], in_=head_dram(kv, 0)[:, s2])
            for b in range(NBLK):
                junk = jpool.tile([P, TPB, P], dt.float32, tag="junkA")
                for j in range(2):
                    t = b * TPB + j
                    nc.scalar.activation(out=junk[:, j], in_=xa[b][:, j],
                                         func=Act.Square,
                                         accum_out=ssq[:, t:t + 1])
                # j=2,3 squared + reduced on GPSIMD to unload ACT and DVE
                nc.gpsimd.tensor_tensor(out=junk[:, 2:], in0=xa[b][:, 2:],
                                        in1=xa[b][:, 2:], op=Alu.mult)
                nc.vector.tensor_reduce(out=ssq[:, b * TPB + 2:b * TPB + 4],
                                        in_=junk[:, 2:],
                                        axis=mybir.AxisListType.X, op=Alu.add)
            # prefetch the first PREF delta-head blocks while anchor computes
            for i in range(PREF // 2):
                load_hd(1 + i // NG, i % NG)
            # stats: an = sqrt(ssq) (Newton-refined), inv = 1/(an+EPS)
            nc.vector.tensor_scalar(out=inv, in0=ssq, scalar1=1e-35, scalar2=None,
                                    op0=Alu.max)
            _sqrt_refined(nc, spool, inv, an, NT)
            nc.vector.tensor_scalar(out=inv, in0=an, scalar1=_f32(EPS), scalar2=None,
                                    op0=Alu.add)
            nc.vector.reciprocal(out=inv, in_=inv)

            yhr = {}

            # ---- anchor: fwd+quantize (DVE-heavy) interleaved with
            # bwd (PE-heavy) at a 2-block stagger ----
            def anchor_fwd(b):
                sl = slice(b * TPB, (b + 1) * TPB)
                pXa = psum.tile([P, TPB, P], dt.float32, tag="psT", bufs=2,
                                name="pXa")
                for j in range(TPB):
                    nc.tensor.transpose(pXa[:, j], xa[b][:, j], id_f)
                xaT = wpool.tile([P, TPB, P], dt.float32, tag="xaT")
                nc.scalar.activation(out=xaT, in_=pXa, func=Act.Copy)
                yP = psum.tile([P, TPB, P], dt.float32, tag="psM",
                               bufs=(2 if sym else 1), name="yP")
                for j in range(TPB):
                    nc.tensor.matmul(yP[:, j], lhsT=xaT[:, j], rhs=r_a_t,
                                     start=True, stop=True)
                yt = wpool.tile([P, TPB, P], dt.float32, tag="yt")
                inv_bc = inv[:, sl][:, :, None].broadcast_to([P, TPB, P])
                nc.vector.tensor_tensor(out=yt, in0=yP, in1=inv_bc, op=Alu.mult)
                q0 = wpool.tile([P, TPB, P], dt.float32, tag="q0")
                if sign_stair:
                    # q0 = K + a1*s1 + a2*s2 + dl3*H3
                    s1 = wpool.tile([P, TPB, P], dt.float32, tag="a1")
                    s2 = wpool.tile([P, TPB, P], dt.float32, tag="a2")
                    h3 = wpool.tile([P, TPB, P], dt.float32, tag="a3")
                    nc.scalar.activation(out=s1, in_=yt, func=Act.Sign,
                                         bias=_f32(-p["a_ts"][0]))
                    nc.vector.tensor_scalar(
                        out=s2, in0=yt, scalar1=p["a_ts"][1], scalar2=None,
                        op0=(Alu.is_ge if p["a_ge"][1] else Alu.is_gt))
                    nc.scalar.activation(out=h3, in_=yt, func=Act.Sign,
                                         bias=_f32(-p["a_ts"][2]))
                    u = wpool.tile([P, TPB, P], dt.float32, tag="u")
                    nc.vector.scalar_tensor_tensor(out=u, in0=s1,
                                                   scalar=p["s_r12"], in1=s2,
                                                   op0=Alu.mult, op1=Alu.add)
                    nc.vector.scalar_tensor_tensor(out=u, in0=u,
                                                   scalar=p["s_r23"], in1=h3,
                                                   op0=Alu.mult, op1=Alu.add)
                    nc.scalar.activation(out=q0, in_=u, func=Act.Copy,
                                         scale=p["s_a3"], bias=p["s_K"])
                else:
                    a1 = wpool.tile([P, TPB, P], dt.float32, tag="a1")
                    a2 = wpool.tile([P, TPB, P], dt.float32, tag="a2")
                    a3 = wpool.tile([P, TPB, P], dt.float32, tag="a3")
                    stair = list(zip(p["a_ts"], p["a_ge"], p["a_dl"]))
                    for ai, (tt, ge, dl) in zip((a1, a2, a3), stair):
                        nc.vector.tensor_scalar(out=ai, in0=yt, scalar1=tt,
                                                scalar2=dl,
                                                op0=(Alu.is_ge if ge else Alu.is_gt),
                                                op1=Alu.mult)
                    nc.vector.scalar_tensor_tensor(out=q0, in0=a1,
                                                   scalar=p["a_c0"], in1=a2,
                                                   op0=Alu.add, op1=Alu.add)
                    nc.vector.tensor_tensor(out=q0, in0=q0, in1=a3, op=Alu.add)
                res = wpool.tile([P, TPB, P], dt.float32, tag="res")
                nc.gpsimd.tensor_tensor(out=res, in0=yt, in1=q0, op=Alu.subtract)
                nc.vector.tensor_reduce(out=alph[:, sl], in_=res,
                                        axis=mybir.AxisListType.X, op=Alu.add,
                                        apply_absolute_value=True)
                nc.vector.tensor_scalar(out=alph[:, sl], in0=alph[:, sl],
                                        scalar1=_f32(1.0 / D), scalar2=None,
                                        op0=Alu.mult)
                sgn = wpool.tile([P, TPB, P], dt.float32, tag="sgn")
                nc.scalar.activation(out=sgn, in_=res, func=Act.Sign)
                al_bc = alph[:, sl][:, :, None].broadcast_to([P, TPB, P])
                nc.gpsimd.tensor_tensor(out=sgn, in0=sgn, in1=al_bc, op=Alu.mult)
                yhr[b] = ypool.tile([P, TPB, P], dt.float32, tag="yh",
                                    name=f"yh{b}")
                nc.gpsimd.tensor_tensor(out=yhr[b], in0=sgn, in1=q0, op=Alu.add)

            def anchor_bwd(b):
                sl = slice(b * TPB, (b + 1) * TPB)
                pYh = psum.tile([P, TPB, P], dt.float32, tag="psT", bufs=2,
                                name="pYh")
                for j in range(TPB):
                    nc.tensor.transpose(pYh[:, j], yhr[b][:, j], id_f)
                del yhr[b]
                yhT = wpool.tile([P, TPB, P], dt.float32, tag="yhT")
                nc.scalar.activation(out=yhT, in_=pYh, func=Act.Copy)
                arP = psum.tile([P, TPB, P], dt.float32, tag="psM",
                                bufs=(2 if sym else 1), name="arP")
                for j in range(TPB):
                    nc.tensor.matmul(arP[:, j], lhsT=yhT[:, j], rhs=r_a,
                                     start=True, stop=True)
                an_bc = an[:, sl][:, :, None].broadcast_to([P, TPB, P])
                nc.vector.tensor_tensor(out=ar[b], in0=arP, in1=an_bc,
                                        op=Alu.mult)
                nc.sync.dma_start(out=head_dram(out, 0)[:, sl], in_=ar[b])

            # ================= DELTA HEADS =================
            # software-pipelined: pass1 of head h+1 interleaves with pass2 of
            # head h at block granularity so ACT/DVE/Pool queues stay fed
            # across the per-head stats barrier.
            nload = NG * (H - 1)

            def load_idx(i):
                if i < nload:
                    load_hd(1 + i // NG, i % NG)

            st = {}

            def head_state(h):
                st[h] = dict(
                    dn2h=hpool.tile([P, NT], dt.bfloat16, tag="dn2h",
                                    name=f"dn2h{h}"),
                    sT=([stpool.tile([P, TPG, P], dt.bfloat16, tag=f"sT{g}",
                                     name=f"sT{h}_{g}") for g in range(NG)]
                        if sym else None),
                    sres=[None] * NG,
                )

            def pass1(h, g):
                v = st[h]
                sl2 = slice(g * TPG, (g + 1) * TPG)
                load_idx((h - 1) * NG + g + PREF // 2)
                hd = hd_tiles.pop((h, g))
                d_t = dpool.tile([P, TPG, P], dt.float32, tag="d")
                nc.gpsimd.tensor_tensor(out=d_t, in0=hd, in1=arh[:, sl2],
                                        op=Alu.subtract)
                dsq2 = jpool.tile([P, TPG, P], dt.bfloat16, tag="dsq2")
                nc.scalar.activation(out=dsq2, in_=d_t, func=Act.Square)
                with nc.allow_low_precision("dn2 bf16 seg-reduce"):
                    nc.vector.tensor_reduce(out=v["dn2h"][:, sl2], in_=dsq2,
                                            axis=mybir.AxisListType.X,
                                            op=Alu.add)
                dT = dtpool.tile([P, TPG, P], dt.float32r, tag="dT")
                if not sym:
                    v["sres"][g] = stpool.tile([P, TPG, P], dt.float32,
                                               tag=f"sr{g}", name=f"sr{h}_{g}")
                for half in range(2):
                    hs = slice(half * TPB, (half + 1) * TPB)
                    pTd = psum.tile([P, TPB, P], dt.float32, tag="psT2",
                                    bufs=(2 if sym else 1), name="pTd")
                    for j in range(TPB):
                        nc.tensor.transpose(pTd[:, j], d_t[:, half * TPB + j],
                                            id_f)
                    nc.scalar.activation(out=dT[:, hs], in_=pTd, func=Act.Copy)
                    zP = psum.tile([P, TPB, P], dt.float32, tag="psM",
                                   bufs=(2 if sym else 1), name="zP")
                    nc.tensor.matmul(zP.rearrange("p j d -> p (j d)"),
                                     lhsT=r_d_t_r,
                                     rhs=dT[:, hs].rearrange("p j d -> p (j d)"),
                                     start=True, stop=True)
                    if sym:
                        nc.scalar.activation(out=v["sT"][g][:, hs], in_=zP,
                                             func=Act.Sign)
                    else:
                        nc.scalar.activation(out=v["sres"][g][:, hs], in_=zP,
                                             func=Act.Copy)

            def head_stats(h):
                v = st[h]
                dnm = hpool.tile([P, NT], dt.float32, tag="dnm", name=f"dnm{h}")
                dn_t = hpool.tile([P, NT], dt.float32, tag="dn_t", name=f"dn{h}")
                dnh = hpool.tile([P, NT], dt.float32, tag="dnh", name=f"dnh{h}")
                nc.vector.tensor_scalar(out=dnm, in0=v["dn2h"], scalar1=1e-35,
                                        scalar2=None, op0=Alu.max)
                _sqrt_refined(nc, hpool, dnm, dn_t, NT)
                if sym:
                    nc.vector.tensor_scalar(out=dnh, in0=dn_t,
                                            scalar1=p["d_h_eff"],
                                            scalar2=None, op0=Alu.mult)
                else:
                    # thr = k2*(dn+eps); sign(z*k1 - thr); scale recon by dn
                    thr = hpool.tile([P, NT], dt.float32, tag="thr",
                                     name=f"thr{h}")
                    nc.vector.tensor_scalar(out=thr, in0=dn_t,
                                            scalar1=_f32(EPS),
                                            scalar2=_f32(p["d_k2"]),
                                            op0=Alu.add, op1=Alu.mult)
                    nc.vector.tensor_scalar(out=dnh, in0=dn_t,
                                            scalar1=_f32(p["d_h_raw"]),
                                            scalar2=None, op0=Alu.mult)
                    v["thr"] = thr
                v["dn_t"], v["dnh"] = dn_t, dnh

            def pass2(h, g):
                v = st[h]
                sl2 = slice(g * TPG, (g + 1) * TPG)
                drain = (h == H - 1)
                ob2 = obpool.tile([P, TPG, P], dt.float32, tag="ob2",
                                  name=f"ob2_{h}_{g}")
                for half in range(2):
                    hs = slice(half * TPB, (half + 1) * TPB)
                    slh = slice(g * TPG + half * TPB,
                                g * TPG + (half + 1) * TPB)
                    if not sym:
                        # broadcast thr along partitions via K=1 matmul, then
                        # sign(z*k1 - thr) per tile (cold path)
                        pThr = psum.tile([TPB, P], dt.float32, tag="psS",
                                         bufs=1, name="pThr")
                        nc.tensor.transpose(pThr, v["thr"][:, slh], id_f)
                        thrs = wpool.tile([TPB, P], dt.float32, tag="thrs")
                        nc.scalar.activation(out=thrs, in_=pThr, func=Act.Copy)
                        sg = wpool.tile([P, TPB, P], dt.float32, tag="sg")
                        for j in range(TPB):
                            thrB = psum.tile([P, P], dt.float32, tag="psB",
                                             bufs=1, name="thrB")
                            nc.tensor.matmul(thrB,
                                             lhsT=onehot4[:, j * P:(j + 1) * P],
                                             rhs=thrs,
                                             start=True, stop=True)
                            nc.vector.scalar_tensor_tensor(
                                out=sg[:, j],
                                in0=v["sres"][g][:, half * TPB + j],
                                scalar=_f32(p["d_k1"]), in1=thrB,
                                op0=Alu.mult, op1=Alu.subtract)
                        qsc = wpool.tile([P, TPB, P], dt.bfloat16, tag="qsc")
                        nc.scalar.activation(out=qsc, in_=sg, func=Act.Sign)
                        # q = m + h_raw * s  (bf16), recon scaled by dn
                        nc.vector.tensor_scalar(out=qsc, in0=qsc,
                                                scalar1=_f32(p["d_h_raw"]),
                                                scalar2=_f32(p["d_m"]),
                                                op0=Alu.mult, op1=Alu.add)
                        qsrc, scale_t = qsc, v["dn_t"]
                        qsl = slice(0, TPB)
                    else:
                        qsrc, scale_t = v["sT"][g], v["dnh"]
                        qsl = hs
                    oP = psum.tile([P, TPB, P], dt.float32, tag="psO",
                                   bufs=2, name="oP")
                    for j in range(TPB):
                        nc.tensor.matmul(oP[:, j],
                                         lhsT=qsrc[:, qsl][:, j],
                                         rhs=r_d_bf, start=True, stop=True)
                    sc_bc = scale_t[:, slh][:, :, None].broadcast_to(
                        [P, TPB, P])
                    nc.vector.tensor_tensor(out=ob2[:, hs], in0=oP, in1=sc_bc,
                                            op=Alu.mult)
                if drain:
                    # split the +anchor add DVE/GPSIMD to thin the tail
                    nc.vector.tensor_tensor(out=ob2[:, :TPB],
                                            in0=ob2[:, :TPB],
                                            in1=arh[:, sl2][:, :TPB],
                                            op=Alu.add)
                    nc.gpsimd.tensor_tensor(out=ob2[:, TPB:],
                                            in0=ob2[:, TPB:],
                                            in1=arh[:, sl2][:, TPB:],
                                            op=Alu.add)
                else:
                    nc.vector.tensor_tensor(out=ob2, in0=ob2,
                                            in1=arh[:, sl2], op=Alu.add)
                nc.sync.dma_start(out=head_dram(out, h)[:, sl2], in_=ob2)

            head_state(1)
            for b in range(NBLK):
                anchor_fwd(b)
            for b in range(NBLK):
                anchor_bwd(b)
            for g in range(NG):
                pass1(1, g)
            head_stats(1)
            # head h+1's pass1 + stats emit before head h's pass2 batch, so
            # the stats chain latency hides under pass2 instead of stalling
            # the next head's output evacuation (psO backpressure -> PE).
            for h in range(1, H):
                if h + 1 < H:
                    head_state(h + 1)
                    for g in range(NG):
                        pass1(h + 1, g)
                    head_stats(h + 1)
                for g in range(NG):
                    pass2(h, g)
                del st[h]
    nc.finalize()
    return nc


SHARED_KEYS = ("r_a_t", "r_a", "r_d_t", "r_d_bf", "ident_f", "ones1", "onehot4")


def core_inputs(shared, kv_states, c):
    return dict(shared, kv=kv_states[c])


def assemble_output(results):
    return np.stack([results[c]["out"] for c in range(B)], axis=0).astype(np.float32)


def kernel(**inputs):
    kv_states = np.ascontiguousarray(np.asarray(inputs["kv_states"], np.float32))
    p = host_prep(inputs["R_anchor"], inputs["cb_anchor"],
                  inputs["R_delta"], inputs["cb_delta"])
    key = (p["d_sym"], p["stair_sign_ok"], tuple(p["a_ts"]), tuple(p["a_ge"]),
           tuple(p["a_dl"]), p["a_c0"], p["d_m"], p["d_h_eff"], p["d_k1"],
           p["d_k2"])
    if key not in _CACHE:
        _CACHE[key] = build(p)
    nc = _CACHE[key]

    shared = {k: p[k] for k in SHARED_KEYS}
    in_maps = [core_inputs(shared, kv_states, c) for c in range(B)]
    res = run_bass_kernel_spmd(nc, in_maps, core_ids=list(range(B)))
    return assemble_output(res.results)


if __name__ == "__main__":
    rng = np.random.default_rng(0)
    fake = {
        "kv_states": rng.standard_normal((B, H, S, D), dtype=np.float32),
        "R_anchor": rng.standard_normal((D, D), dtype=np.float32),
        "cb_anchor": np.sort(rng.standard_normal(4).astype(np.float32)),
        "R_delta": rng.standard_normal((D, D), dtype=np.float32),
        "cb_delta": np.sort(rng.standard_normal(2).astype(np.float32)),
    }
    o = kernel(**fake)
    print("ran", o.shape, o.dtype)

